# revision 1
# baseline (speedup 1.0000x reference)
"""Distributed causal single-head attention kernel for 8 TRN2 NeuronCores.

Problem (hardcoded): x [4, 2048, 1024], Wq/Wk/Wv [1024, 1024] (torch Linear
layout, y = x @ W.T), causal softmax attention, out [4, 2048, 1024] f32.

K and V are never materialized; the weight projections are reassociated
onto the query side, which eliminates the key-side duplication across the
two cores sharing a batch:
    R   = (Wq^T Wk)^T-applied queries   (R-proj, G folded on host)
    S^T = x_kd @ R                      (scores, transposed)
    Z   = x_kd^T @ P                    (key contraction)
    O^T = Wv @ Z                        (Wv applied after)
8 cores = 4 batches x 2 cores, folded q-block pairing {0,3}/{1,2}.

Performance structure (PE streams ~0.5 ns/column on this part, so wall
time is dominated by streamed matmul columns):
  - causal width trimming: diagonal score tiles (GA kb=1..3, GB kb=13..15)
    only stream the causally-live q columns (384/256/128 wide); matching
    trims in Z chains, rowsum chains, exp and mask ops. Chains put the
    full-width element first so PSUM has_written covers the whole tile.
  - all tile pools are root-scoped and rotate via tags, so the per-rep
    pool teardown Drain barriers disappear; PSUM budget (8 banks):
    R/O share a 2-buf tag, scores 3 bufs, Z 2, rowsum 1.
  - output DMAs ride the Activation HWDGE queue so the next body's input
    DMAs on the SP queue are not head-of-line blocked behind them.
  - flex combines use fused scalar_tensor_tensor (1 DVE op instead of 2).
"""

import sys
import numpy as np

for _p in ("/opt/trn_rl_repo",):
    if _p not in sys.path:
        sys.path.insert(0, _p)

import ml_dtypes

B, S, D = 4, 2048, 1024
QB = 512          # q-tile width (2 per core)
KB = 128          # key block
NKB = S // KB     # 16 key blocks
ND = D // 128     # 8 d-slices
QPOS = (0, 1536)  # positions of the two q blocks inside the gathered key axis
GA = tuple(range(0, 4))
GF = tuple(range(4, 8))
GB = tuple(range(0, 4)) + tuple(range(8, 16))
N_CORES = 8

_SCALE = 1.0 / float(np.sqrt(np.float32(D)))


def _trim(kb_rel):
    return 128 * kb_rel


def _core_layout(core):
    """(batch, [qblock row-block indices], key gather order, wa)."""
    b, t = core // 2, core % 2
    if t == 0:
        qbs = [0, 3]
        order = [0, 1, 2, 3]
        wa = 0.0
    else:
        qbs = [1, 2]
        order = [1, 0, 0, 2]
        wa = 1.0
    gather = np.concatenate([np.arange(o * QB, (o + 1) * QB) for o in order])
    return b, qbs, gather, wa


def build_nc(body_reps=1):
    """Build the SPMD Bass graph (same program for all 8 cores)."""
    import concourse.tile as tile
    import concourse.mybir as mybir
    from concourse import bacc
    from contextlib import ExitStack

    fp32 = mybir.dt.float32
    bf16 = mybir.dt.bfloat16
    MUL = mybir.AluOpType.mult
    ADD = mybir.AluOpType.add

    nc = bacc.Bacc("TRN2", target_bir_lowering=False, debug=False)

    xT = nc.dram_tensor("xT", [D, S], bf16, kind="ExternalInput").ap()
    xkd = nc.dram_tensor("xkd", [S, D], bf16, kind="ExternalInput").ap()
    gT = nc.dram_tensor("gT", [D, D], bf16, kind="ExternalInput").ap()
    wvT = nc.dram_tensor("wvT", [D, D], bf16, kind="ExternalInput").ap()
    masks = nc.dram_tensor("masks", [8, KB, QB], bf16, kind="ExternalInput").ap()
    wsel = nc.dram_tensor("wsel", [KB, 2], fp32, kind="ExternalInput").ap()
    wlsel = nc.dram_tensor("wlsel", [2, QB], fp32, kind="ExternalInput").ap()
    outT = nc.dram_tensor("outT", [D, 2 * QB], fp32, kind="ExternalOutput").ap()
    lsum = nc.dram_tensor("lsum", [2, QB], fp32, kind="ExternalOutput").ap()

    xT_r = xT.rearrange("(a p) s -> a p s", p=128)       # [8, 128, 2048]
    xT_p = xT.rearrange("(a p) s -> p a s", p=128)       # [128, 8, 2048]
    xkd_p = xkd.rearrange("(kb p) d -> p kb d", p=128)   # [128, 16, 1024]
    g_r = gT.rearrange("(a p) d -> a p d", p=128)
    wv_p = wvT.rearrange("(a p) d -> p a d", p=128)
    masks_p = masks.rearrange("k p q -> p k q")          # [128, 8, 512]
    outT_r = outT.rearrange("(a p) q -> a p q", p=128)   # [8, 128, 1024]

    QW = 2 * QB  # 1024 q rows per core

    # score-tile schedule: (key block, r source, p column, mask idx, q0 trim)
    # r source: 0 = R_A, 1 = R_B, 2 = rflex
    SCHED = (
        [(kb, 0, i, i, _trim(i)) for i, kb in enumerate(GA)] +
        [(kb, 2, 4 + i, None, 0) for i, kb in enumerate(GF)] +
        [(kb, 1, 8 + i,
          (4 + kb - 12 if kb >= 12 else None),
          (_trim(kb - 12) if kb >= 12 else 0))
         for i, kb in enumerate(GB)]
    )
    NT = len(SCHED)  # 20

    with tile.TileContext(nc) as tc:
        with ExitStack() as root:
            const = root.enter_context(tc.tile_pool(name="const", bufs=1))
            ones_bf = const.tile([128, 1], bf16)
            nc.vector.memset(ones_bf[:], 1.0)
            ws = const.tile([KB, 2], fp32)
            wls_a = const.tile([1, QB], fp32, tag="wlsa")
            wls_b = const.tile([1, QB], fp32, tag="wlsb")

            persist = root.enter_context(tc.tile_pool(name="persist", bufs=1))
            xt_bf = persist.tile([128, ND * S], bf16, tag="xt")     # 32KB/part
            xkd_bf = persist.tile([128, NKB * D], bf16, tag="xkd")  # 32KB/part
            rt = persist.tile([128, ND * QW], bf16, tag="rt")       # 16KB/part
            rfx = persist.tile([128, ND * QB], bf16, tag="rfx")     # 8KB/part
            mk = persist.tile([128, 8 * QB], bf16, tag="mk")        # 8KB/part
            wv_bf = persist.tile([128, ND * D], bf16, tag="wv")     # 16KB/part
            za = persist.tile([128, ND * QB], bf16, tag="za")       # 8KB/part
            zb = persist.tile([128, ND * QB], bf16, tag="zb")       # 8KB/part
            zf = persist.tile([128, ND * QB], bf16, tag="zf")       # 8KB/part

            # root-scoped working pools (no per-rep teardown drains)
            wp = root.enter_context(tc.tile_pool(name="wbf", bufs=1))
            qtmp = root.enter_context(tc.tile_pool(name="qtmp", bufs=2))
            pp = root.enter_context(tc.tile_pool(name="pp", bufs=1))
            rp = root.enter_context(tc.tile_pool(name="rp", bufs=2))
            op = root.enter_context(tc.tile_pool(name="op", bufs=4))
            # single PSUM pool, per-tag rotation: 2+3+2+1 = 8 banks
            psu = root.enter_context(
                tc.tile_pool(name="psu", bufs=1, space="PSUM"))

            for rep in range(body_reps):
                # ---------- phase A: load, Q-proj, R-proj ----------
                w_bf = wp.tile([128, ND * D], bf16, tag="w")
                xt_v = xt_bf[:].rearrange("p (a s) -> p a s", a=ND)
                nc.sync.dma_start(w_bf[:, 0:128], g_r[0][:, 0:128])
                nc.sync.dma_start(
                    xt_bf[:, QPOS[0]: QPOS[0] + QB],
                    xT_r[0][:, QPOS[0]:QPOS[0] + QB])
                nc.sync.dma_start(w_bf[:, 128:D], g_r[0][:, 128:D])
                for a in range(1, ND):
                    nc.sync.dma_start(w_bf[:, a * D:(a + 1) * D], g_r[a])
                    nc.sync.dma_start(
                        xt_bf[:, a * S + QPOS[0]: a * S + QPOS[0] + QB],
                        xT_r[a][:, QPOS[0]:QPOS[0] + QB])
                for a in range(ND):
                    nc.sync.dma_start(
                        xt_bf[:, a * S + QPOS[1]: a * S + QPOS[1] + QB],
                        xT_r[a][:, QPOS[1]:QPOS[1] + QB])
                nc.sync.dma_start(
                    xt_v[:, :, QB:QB + QB], xT_p[:, :, QB:QB + QB])
                nc.sync.dma_start(
                    xt_v[:, :, 2 * QB:QPOS[1]], xT_p[:, :, 2 * QB:QPOS[1]])
                if rep == 0:
                    nc.sync.dma_start(ws[:], wsel[:])
                    nc.sync.dma_start(wls_a[:], wlsel[0:1, :])
                    nc.sync.dma_start(wls_b[:], wlsel[1:2, :])

                # xkd, wv chunked (consumed much later) — smaller transfers
                # interleave with the previous body's output DMAs instead of
                # hogging a DMA engine for 10+ us.
                xkd_v = xkd_bf[:].rearrange("p (kb d) -> p kb d", kb=NKB)
                for kb4 in range(0, NKB, 4):
                    nc.sync.dma_start(
                        xkd_v[:, kb4:kb4 + 4, :], xkd_p[:, kb4:kb4 + 4, :])
                wv_v = wv_bf[:].rearrange("p (a d) -> p a d", a=ND)
                for a4 in range(0, ND, 4):
                    nc.sync.dma_start(
                        wv_v[:, a4:a4 + 4, :], wv_p[:, a4:a4 + 4, :])
                if rep == 0:
                    nc.sync.dma_start(
                        mk[:].rearrange("p (k q) -> p k q", k=8), masks_p)

                # R [din, q] = G^T @ x_q^T with G = Wk^T Wq folded on host
                for qc in range(2):
                    qp = QPOS[qc]
                    for din in range(ND):
                        pt = psu.tile([128, QB], fp32, tag="w5", bufs=2)
                        for a in range(ND):
                            nc.tensor.matmul(
                                pt[:],
                                w_bf[:, a * D + din * 128: a * D + din * 128 + 128],
                                xt_bf[:, a * S + qp: a * S + qp + QB],
                                start=(a == 0), stop=(a == ND - 1))
                        nc.vector.tensor_copy(
                            rt[:, din * QW + qc * QB: din * QW + qc * QB + QB],
                            pt[:])

                # rflex = wa*R_A + wb*R_B = wa*(R_A - R_B) + R_B
                for a in range(ND):
                    ra = rt[:, a * QW: a * QW + QB]
                    rb = rt[:, a * QW + QB: a * QW + 2 * QB]
                    t1 = qtmp.tile([128, QB], bf16, tag="t1")
                    nc.vector.tensor_sub(t1[:], ra, rb)
                    nc.vector.scalar_tensor_tensor(
                        rfx[:, a * QB:(a + 1) * QB],
                        t1[:], ws[:, 0:1], rb, MUL, ADD)

                # ---------- phase B1: scores + exp + rowsums ----------
                p_bf = pp.tile([128, NT * QB], bf16, tag="p")

                def rsrc_ap(qs, a, q0, q1):
                    if qs == 0:
                        base = a * QW
                    elif qs == 1:
                        base = a * QW + QB
                    else:
                        return rfx[:, a * QB + q0: a * QB + q1]
                    return rt[:, base + q0: base + q1]

                def score_tile(kb, qs, pcol, mi, q0):
                    pst = psu.tile([128, QB], fp32, tag="s", bufs=3)
                    for a in range(ND):
                        nc.tensor.matmul(
                            pst[:, q0:QB],
                            xt_bf[:, a * S + kb * 128: a * S + kb * 128 + 128],
                            rsrc_ap(qs, a, q0, QB),
                            start=(a == 0), stop=(a == ND - 1))
                    pcol_ap = p_bf[:, pcol * QB + q0:(pcol + 1) * QB]
                    nc.scalar.activation(
                        pcol_ap, pst[:, q0:QB],
                        mybir.ActivationFunctionType.Exp,
                        scale=_SCALE)
                    if mi is not None:
                        nc.gpsimd.tensor_mul(
                            pcol_ap, pcol_ap,
                            mk[:, mi * QB + q0:(mi + 1) * QB])

                # rowsum chains; (pcol, q0) with the full-width element first.
                # One PSUM bank (tag "l"); each sum is copied straight out to
                # a small SBUF tile so the bank can be reused by the next
                # chain (la/lf/lb run back to back).
                def rowsum(pcols, tag):
                    plt = psu.tile([1, QB], fp32, tag="l", bufs=1)
                    pcols = list(pcols)
                    for j, (pcol, q0) in enumerate(pcols):
                        nc.tensor.matmul(
                            plt[:, q0:QB], ones_bf[:],
                            p_bf[:, pcol * QB + q0:(pcol + 1) * QB],
                            start=(j == 0), stop=(j == len(pcols) - 1))
                    sb = rp.tile([1, QB], fp32, tag=tag)
                    nc.vector.tensor_copy(sb[:], plt[:])
                    return sb

                # Emission order hides exp/mask latency under PE work:
                # GA+GB score tiles, then la (p cols long ready), then the
                # flex tiles (GB's last exp/mask completes under them), then
                # lb, then lf right after the flex exps.
                for t in SCHED:
                    if t[1] == 0 or (t[1] == 1 and t[3] is None):
                        score_tile(*t)          # GA + GB unmasked
                for t in SCHED:
                    if t[1] == 2:
                        score_tile(*t)          # GF (flex)
                l_a = rowsum([(i, _trim(i)) for i in range(4)], "la")
                for t in SCHED:
                    if t[1] == 1 and t[3] is not None:
                        score_tile(*t)          # GB diag (masked)
                l_b = rowsum(
                    [(i, 0) for i in range(8, 16)] +
                    [(i, _trim(i - 16)) for i in range(16, 20)], "lb")
                l_f = rowsum([(i, 0) for i in range(4, 8)], "lf")

                for s, l_s, wl in ((0, l_a, wls_a), (1, l_b, wls_b)):
                    tl = rp.tile([1, QB], fp32, tag="tl")
                    nc.vector.tensor_mul(tl[:], l_f[:], wl[:])
                    lo = rp.tile([1, QB], fp32, tag="lo")
                    nc.vector.tensor_add(lo[:], l_s[:], tl[:])
                    nc.scalar.dma_start(lsum[s:s + 1, :], lo[:])

                # ------- phase B2: Z = x^T P (+combine), O = Wv Z -------
                def zchain(elems, dsl):
                    """elems: [(pcol, kb, q0)], full width first."""
                    pz = psu.tile([128, QB], fp32, tag="z", bufs=2)
                    for j, (pcol, kb, q0) in enumerate(elems):
                        nc.tensor.matmul(
                            pz[:, q0:QB],
                            xkd_bf[:, kb * D + dsl * 128: kb * D + dsl * 128 + 128],
                            p_bf[:, pcol * QB + q0:(pcol + 1) * QB],
                            start=(j == 0), stop=(j == len(elems) - 1))
                    return pz

                ZF = [(4 + i, kb, 0) for i, kb in enumerate(GF)]
                ZA = [(i, kb, _trim(i)) for i, kb in enumerate(GA)]
                ZB = ([(8 + i, kb, 0) for i, kb in enumerate(GB[:8])] +
                      [(16 + i, kb, _trim(i)) for i, kb in enumerate(GB[8:])])

                # O^T [dv, q] = Wv @ Z (unnormalized; host divides)
                def ochain(s, zt):
                    for dv in range(ND):
                        po = psu.tile([128, QB], fp32, tag="w5", bufs=2)
                        for a in range(ND):
                            nc.tensor.matmul(
                                po[:],
                                wv_bf[:, a * D + dv * 128: a * D + dv * 128 + 128],
                                zt[:, a * QB:(a + 1) * QB],
                                start=(a == 0), stop=(a == ND - 1))
                        ot = op.tile([128, QB], fp32, tag="ot")
                        nc.vector.tensor_copy(ot[:], po[:])
                        nc.scalar.dma_start(
                            outT_r[dv][:, s * QB:(s + 1) * QB], ot[:])

                # Z_F first (plain copy), then A with flex combine, then the
                # O chains for q-block A (they only need za — their ot
                # copies/DMAs drain while the B-side Z chains stream), then
                # B and its O chains.
                for dsl in range(ND):
                    pz = zchain(ZF, dsl)
                    nc.vector.tensor_copy(
                        zf[:, dsl * QB:(dsl + 1) * QB], pz[:])
                for dsl in range(ND):
                    pz = zchain(ZA, dsl)
                    nc.vector.scalar_tensor_tensor(
                        za[:, dsl * QB:(dsl + 1) * QB],
                        zf[:, dsl * QB:(dsl + 1) * QB],
                        ws[:, 0:1], pz[:], MUL, ADD)
                ochain(0, za)
                for dsl in range(ND):
                    pz = zchain(ZB, dsl)
                    nc.vector.scalar_tensor_tensor(
                        zb[:, dsl * QB:(dsl + 1) * QB],
                        zf[:, dsl * QB:(dsl + 1) * QB],
                        ws[:, 1:2], pz[:], MUL, ADD)
                ochain(1, zb)

    nc.compile()
    return nc


_NC_CACHE = {}


def _get_nc(body_reps=1):
    if body_reps not in _NC_CACHE:
        _NC_CACHE[body_reps] = build_nc(body_reps)
    return _NC_CACHE[body_reps]


def make_in_maps(x, Wq, Wk, Wv):
    """Host-side sharding: per-core input dict."""
    x = np.asarray(x, dtype=np.float32)
    gTn = np.ascontiguousarray(
        np.asarray(Wq, np.float64).T @ np.asarray(Wk, np.float64)
    ).astype(ml_dtypes.bfloat16)
    wvT = np.ascontiguousarray(np.asarray(Wv, np.float32).T).astype(ml_dtypes.bfloat16)

    in_maps = []
    for core in range(N_CORES):
        b, qbs, gather, wa = _core_layout(core)
        xg = x[b][gather]                                    # [S, D] gathered
        xkd = np.ascontiguousarray(xg).astype(ml_dtypes.bfloat16)
        xTp = np.ascontiguousarray(xg.T).astype(ml_dtypes.bfloat16)
        mkd = np.zeros((8, KB, QB), np.float32)
        for i in range(4):      # GA diag masks (key blocks 0..3 vs q-block A)
            krows = gather[i * KB:(i + 1) * KB][:, None]
            qrows = (qbs[0] * QB + np.arange(QB))[None, :]
            mkd[i] = (krows <= qrows)
        for j in range(4):      # GB key blocks 12..15 vs q-block B
            krows = gather[(12 + j) * KB:(13 + j) * KB][:, None]
            qrows = (qbs[1] * QB + np.arange(QB))[None, :]
            mkd[4 + j] = (krows <= qrows)
        wsel = np.zeros((KB, 2), np.float32)
        wsel[:, 0] = wa
        wsel[:, 1] = 1.0 - wa
        wlsel = np.zeros((2, QB), np.float32)
        wlsel[0, :] = wa
        wlsel[1, :] = 1.0 - wa
        in_maps.append({
            "xT": xTp,
            "xkd": xkd,
            "gT": gTn,
            "wvT": wvT,
            "masks": mkd.astype(ml_dtypes.bfloat16),
            "wsel": wsel,
            "wlsel": wlsel,
        })
    return in_maps


def assemble_output(results):
    out = np.zeros((B, S, D), np.float32)
    for core in range(N_CORES):
        b, qbs, _, _ = _core_layout(core)
        outT = results[core]["outT"]      # [D, 1024] unnormalized
        l = results[core]["lsum"]         # [2, QB]
        for slot in range(2):
            rows = np.arange(qbs[slot] * QB, (qbs[slot] + 1) * QB)
            o = outT[:, slot * QB:(slot + 1) * QB].T   # [QB, D]
            out[b, rows, :] = o / l[slot][:, None]
    return out


def kernel(x, Wq, Wk, Wv):
    from concourse.bass_utils import run_bass_kernel_spmd
    nc = _get_nc()
    in_maps = make_in_maps(x, Wq, Wk, Wv)
    res = run_bass_kernel_spmd(nc, in_maps, core_ids=list(range(N_CORES)))
    return assemble_output(res.results)



# revision 2
# speedup vs baseline: 1.0151x; 1.0151x over previous
"""Distributed causal single-head attention kernel for 8 TRN2 NeuronCores.

Same algorithm as v1 (G-folded score reassociation, K/V never materialized,
8 cores = 4 batches x 2, folded q-block pairing, causal width trimming), with
a restructured PE schedule:

  - chains that share a stationary operand are interleaved pairwise so the
    PE weight path (LDWEIGHTS/FWL) sees each weight once per pair:
      R-proj: qc=0/qc=1 chains per (din) share G slices
      scores: GA kb / GB kb (kb 0..3) share xT key slices
      Z: ZA/ZB links for kb 0..3 share xkd slices
      O-proj: q-block A/B chains share Wv slices
  - all Z chains (and their DVE combines) complete before the O phase, so
    O-chain links never wait on trailing combines; O A/B interleaving then
    overlaps the 16 output copies/DMAs with the whole O phase.
  - PSUM tags: pairs allocate 2 banks in flight; Z-phase B-chains borrow the
    score banks (idle in that phase); the first R pair borrows them at body
    start so it does not wait for the previous rep's O drains.
"""

import sys
import numpy as np

for _p in ("/opt/trn_rl_repo",):
    if _p not in sys.path:
        sys.path.insert(0, _p)

import ml_dtypes

B, S, D = 4, 2048, 1024
QB = 512
KB = 128
NKB = S // KB
ND = D // 128
QPOS = (0, 1536)
GA = tuple(range(0, 4))
GF = tuple(range(4, 8))
GB = tuple(range(0, 4)) + tuple(range(8, 16))
N_CORES = 8

_SCALE = 1.0 / float(np.sqrt(np.float32(D)))


def _trim(kb_rel):
    return 128 * kb_rel


def _core_layout(core):
    b, t = core // 2, core % 2
    if t == 0:
        qbs = [0, 3]
        order = [0, 1, 2, 3]
        wa = 0.0
    else:
        qbs = [1, 2]
        order = [1, 0, 0, 2]
        wa = 1.0
    gather = np.concatenate([np.arange(o * QB, (o + 1) * QB) for o in order])
    return b, qbs, gather, wa


def build_nc(body_reps=1):
    import concourse.tile as tile
    import concourse.mybir as mybir
    from concourse import bacc
    from contextlib import ExitStack

    fp32 = mybir.dt.float32
    bf16 = mybir.dt.bfloat16
    MUL = mybir.AluOpType.mult
    ADD = mybir.AluOpType.add

    nc = bacc.Bacc("TRN2", target_bir_lowering=False, debug=False)

    xT = nc.dram_tensor("xT", [D, S], bf16, kind="ExternalInput").ap()
    xkd = nc.dram_tensor("xkd", [S, D], bf16, kind="ExternalInput").ap()
    gT = nc.dram_tensor("gT", [D, D], bf16, kind="ExternalInput").ap()
    wvT = nc.dram_tensor("wvT", [D, D], bf16, kind="ExternalInput").ap()
    masks = nc.dram_tensor("masks", [8, KB, QB], bf16, kind="ExternalInput").ap()
    wsel = nc.dram_tensor("wsel", [KB, 2], fp32, kind="ExternalInput").ap()
    wlsel = nc.dram_tensor("wlsel", [2, QB], fp32, kind="ExternalInput").ap()
    outT = nc.dram_tensor("outT", [D, 2 * QB], fp32, kind="ExternalOutput").ap()
    lsum = nc.dram_tensor("lsum", [2, QB], fp32, kind="ExternalOutput").ap()

    xT_r = xT.rearrange("(a p) s -> a p s", p=128)
    xT_p = xT.rearrange("(a p) s -> p a s", p=128)
    xkd_p = xkd.rearrange("(kb p) d -> p kb d", p=128)
    g_r = gT.rearrange("(a p) d -> a p d", p=128)
    wv_p = wvT.rearrange("(a p) d -> p a d", p=128)
    masks_p = masks.rearrange("k p q -> p k q")
    outT_r = outT.rearrange("(a p) q -> a p q", p=128)

    QW = 2 * QB

    with tile.TileContext(nc) as tc:
        with ExitStack() as root:
            const = root.enter_context(tc.tile_pool(name="const", bufs=1))
            ones_bf = const.tile([128, 1], bf16)
            nc.vector.memset(ones_bf[:], 1.0)
            ws = const.tile([KB, 2], fp32)
            wls_a = const.tile([1, QB], fp32, tag="wlsa")
            wls_b = const.tile([1, QB], fp32, tag="wlsb")

            persist = root.enter_context(tc.tile_pool(name="persist", bufs=1))
            xt_bf = persist.tile([128, ND * S], bf16, tag="xt")
            xkd_bf = persist.tile([128, NKB * D], bf16, tag="xkd")
            rt = persist.tile([128, ND * QW], bf16, tag="rt")
            rfx = persist.tile([128, ND * QB], bf16, tag="rfx")
            mk = persist.tile([128, 8 * QB], bf16, tag="mk")
            wv_bf = persist.tile([128, ND * D], bf16, tag="wv")
            za = persist.tile([128, ND * QB], bf16, tag="za")
            zb = persist.tile([128, ND * QB], bf16, tag="zb")
            zf = persist.tile([128, ND * QB], bf16, tag="zf")

            wp = root.enter_context(tc.tile_pool(name="wbf", bufs=1))
            qtmp = root.enter_context(tc.tile_pool(name="qtmp", bufs=2))
            pp = root.enter_context(tc.tile_pool(name="pp", bufs=1))
            rp = root.enter_context(tc.tile_pool(name="rp", bufs=2))
            op = root.enter_context(tc.tile_pool(name="op", bufs=4))
            psu = root.enter_context(
                tc.tile_pool(name="psu", bufs=1, space="PSUM"))

            for rep in range(body_reps):
                # ---------- phase A: load, R-proj ----------
                w_bf = wp.tile([128, ND * D], bf16, tag="w")
                xt_v = xt_bf[:].rearrange("p (a s) -> p a s", a=ND)
                nc.sync.dma_start(w_bf[:, 0:128], g_r[0][:, 0:128])
                nc.sync.dma_start(
                    xt_bf[:, QPOS[0]: QPOS[0] + QB],
                    xT_r[0][:, QPOS[0]:QPOS[0] + QB])
                nc.sync.dma_start(w_bf[:, 128:D], g_r[0][:, 128:D])
                for a in range(1, ND):
                    nc.sync.dma_start(w_bf[:, a * D:(a + 1) * D], g_r[a])
                    nc.sync.dma_start(
                        xt_bf[:, a * S + QPOS[0]: a * S + QPOS[0] + QB],
                        xT_r[a][:, QPOS[0]:QPOS[0] + QB])
                for a in range(ND):
                    nc.sync.dma_start(
                        xt_bf[:, a * S + QPOS[1]: a * S + QPOS[1] + QB],
                        xT_r[a][:, QPOS[1]:QPOS[1] + QB])
                nc.sync.dma_start(
                    xt_v[:, :, QB:QB + QB], xT_p[:, :, QB:QB + QB])
                nc.sync.dma_start(
                    xt_v[:, :, 2 * QB:QPOS[1]], xT_p[:, :, 2 * QB:QPOS[1]])
                if rep == 0:
                    nc.sync.dma_start(ws[:], wsel[:])
                    nc.sync.dma_start(wls_a[:], wlsel[0:1, :])
                    nc.sync.dma_start(wls_b[:], wlsel[1:2, :])

                xkd_v = xkd_bf[:].rearrange("p (kb d) -> p kb d", kb=NKB)
                for kb4 in range(0, NKB, 4):
                    nc.sync.dma_start(
                        xkd_v[:, kb4:kb4 + 4, :], xkd_p[:, kb4:kb4 + 4, :])
                wv_v = wv_bf[:].rearrange("p (a d) -> p a d", a=ND)
                for a4 in range(0, ND, 4):
                    nc.sync.dma_start(
                        wv_v[:, a4:a4 + 4, :], wv_p[:, a4:a4 + 4, :])
                if rep == 0:
                    nc.sync.dma_start(
                        mk[:].rearrange("p (k q) -> p k q", k=8), masks_p)

                # R [din, q] = G^T @ x_q^T; qc=0/1 chains interleaved so each
                # G slice is loaded once per pair. First pair borrows the
                # score banks (free at body start).
                for din in range(ND):
                    # alternate bank pools so each pool has a 2-pair reuse
                    # distance (drain comfortably beats the next claim)
                    tag = "s" if din % 2 == 0 else "w5"
                    bufs = 3 if tag == "s" else 2
                    p0 = psu.tile([128, QB], fp32, tag=tag, bufs=bufs)
                    p1 = psu.tile([128, QB], fp32, tag=tag, bufs=bufs)
                    for a in range(ND):
                        w_ap = w_bf[:, a * D + din * 128: a * D + din * 128 + 128]
                        nc.tensor.matmul(
                            p0[:], w_ap,
                            xt_bf[:, a * S + QPOS[0]: a * S + QPOS[0] + QB],
                            start=(a == 0), stop=(a == ND - 1))
                        nc.tensor.matmul(
                            p1[:], w_ap,
                            xt_bf[:, a * S + QPOS[1]: a * S + QPOS[1] + QB],
                            start=(a == 0), stop=(a == ND - 1))
                    nc.vector.tensor_copy(
                        rt[:, din * QW: din * QW + QB], p0[:])
                    nc.vector.tensor_copy(
                        rt[:, din * QW + QB: din * QW + 2 * QB], p1[:])

                # rflex = wa*(R_A - R_B) + R_B
                for a in range(ND):
                    ra = rt[:, a * QW: a * QW + QB]
                    rb = rt[:, a * QW + QB: a * QW + 2 * QB]
                    t1 = qtmp.tile([128, QB], bf16, tag="t1")
                    nc.vector.tensor_sub(t1[:], ra, rb)
                    nc.vector.scalar_tensor_tensor(
                        rfx[:, a * QB:(a + 1) * QB],
                        t1[:], ws[:, 0:1], rb, MUL, ADD)

                # ---------- phase B1: scores + exp + rowsums ----------
                NT = 20
                p_bf = pp.tile([128, NT * QB], bf16, tag="p")

                def expout(pcol, pst, q0, mi):
                    pcol_ap = p_bf[:, pcol * QB + q0:(pcol + 1) * QB]
                    nc.scalar.activation(
                        pcol_ap, pst[:, q0:QB],
                        mybir.ActivationFunctionType.Exp,
                        scale=_SCALE)
                    if mi is not None:
                        nc.gpsimd.tensor_mul(
                            pcol_ap, pcol_ap,
                            mk[:, mi * QB + q0:(mi + 1) * QB])

                # paired GA/GB tiles for kb 0..3 (shared xT key slices)
                for i, kb in enumerate(GA):
                    q0 = _trim(i)
                    pA = psu.tile([128, QB], fp32, tag="s", bufs=3)
                    pB = psu.tile([128, QB], fp32, tag="s", bufs=3)
                    for a in range(ND):
                        xk_ap = xt_bf[:, a * S + kb * 128: a * S + kb * 128 + 128]
                        nc.tensor.matmul(
                            pA[:, q0:QB], xk_ap,
                            rt[:, a * QW + q0: a * QW + QB],
                            start=(a == 0), stop=(a == ND - 1))
                        nc.tensor.matmul(
                            pB[:], xk_ap,
                            rt[:, a * QW + QB: a * QW + 2 * QB],
                            start=(a == 0), stop=(a == ND - 1))
                    expout(i, pA, q0, i)
                    expout(8 + i, pB, 0, None)

                # GF flex tiles (kb 4..7, rflex moving)
                for i, kb in enumerate(GF):
                    pF = psu.tile([128, QB], fp32, tag="s", bufs=3)
                    for a in range(ND):
                        nc.tensor.matmul(
                            pF[:],
                            xt_bf[:, a * S + kb * 128: a * S + kb * 128 + 128],
                            rfx[:, a * QB:(a + 1) * QB],
                            start=(a == 0), stop=(a == ND - 1))
                    expout(4 + i, pF, 0, None)

                def rowsum(pcols, tag):
                    plt = psu.tile([1, QB], fp32, tag="l", bufs=1)
                    pcols = list(pcols)
                    for j, (pcol, q0) in enumerate(pcols):
                        nc.tensor.matmul(
                            plt[:, q0:QB], ones_bf[:],
                            p_bf[:, pcol * QB + q0:(pcol + 1) * QB],
                            start=(j == 0), stop=(j == len(pcols) - 1))
                    sb = rp.tile([1, QB], fp32, tag=tag)
                    nc.vector.tensor_copy(sb[:], plt[:])
                    return sb

                # GB far tiles (kb 8..15, pcol 12..19; 12..15 diag-masked)
                for j, kb in enumerate(GB[4:]):
                    mi = (4 + kb - 12) if kb >= 12 else None
                    q0 = _trim(kb - 12) if kb >= 12 else 0
                    pB = psu.tile([128, QB], fp32, tag="s", bufs=3)
                    for a in range(ND):
                        nc.tensor.matmul(
                            pB[:, q0:QB],
                            xt_bf[:, a * S + kb * 128: a * S + kb * 128 + 128],
                            rt[:, a * QW + QB + q0: a * QW + 2 * QB],
                            start=(a == 0), stop=(a == ND - 1))
                    expout(12 + j, pB, q0, mi)

                # all three rowsum chains back-to-back: 20 consecutive
                # ones-stationary links keep the PE weight path warm
                l_a = rowsum([(i, _trim(i)) for i in range(4)], "la")
                l_b = rowsum(
                    [(i, 0) for i in range(8, 16)] +
                    [(i, _trim(i - 16)) for i in range(16, 20)], "lb")
                l_f = rowsum([(i, 0) for i in range(4, 8)], "lf")

                for s, l_s, wl in ((0, l_a, wls_a), (1, l_b, wls_b)):
                    tl = rp.tile([1, QB], fp32, tag="tl")
                    nc.vector.tensor_mul(tl[:], l_f[:], wl[:])
                    lo = rp.tile([1, QB], fp32, tag="lo")
                    nc.vector.tensor_add(lo[:], l_s[:], tl[:])
                    nc.scalar.dma_start(lsum[s:s + 1, :], lo[:])

                # ------- phase B2: Z = x^T P (+combine) -------
                # ZF first (plain copies), then per dsl the ZA/ZB pair with
                # kb 0..3 links interleaved (shared xkd slices). pzB borrows
                # the score banks (idle in this phase).
                for dsl in range(ND):
                    pz = psu.tile([128, QB], fp32, tag="z", bufs=2)
                    for j, kb in enumerate(GF):
                        nc.tensor.matmul(
                            pz[:],
                            xkd_bf[:, kb * D + dsl * 128: kb * D + dsl * 128 + 128],
                            p_bf[:, (4 + j) * QB:(5 + j) * QB],
                            start=(j == 0), stop=(j == 3))
                    nc.vector.tensor_copy(
                        zf[:, dsl * QB:(dsl + 1) * QB], pz[:])

                for dsl in range(ND):
                    pzA = psu.tile([128, QB], fp32, tag="z", bufs=2)
                    pzB = psu.tile([128, QB], fp32, tag="s", bufs=3)
                    for i, kb in enumerate(GA):
                        q0 = _trim(i)
                        xk_ap = xkd_bf[:, kb * D + dsl * 128: kb * D + dsl * 128 + 128]
                        nc.tensor.matmul(
                            pzA[:, q0:QB], xk_ap,
                            p_bf[:, i * QB + q0:(i + 1) * QB],
                            start=(i == 0), stop=(i == 3))
                        nc.tensor.matmul(
                            pzB[:], xk_ap,
                            p_bf[:, (8 + i) * QB:(9 + i) * QB],
                            start=(i == 0), stop=False)
                    nc.vector.scalar_tensor_tensor(
                        za[:, dsl * QB:(dsl + 1) * QB],
                        zf[:, dsl * QB:(dsl + 1) * QB],
                        ws[:, 0:1], pzA[:], MUL, ADD)
                    for j, kb in enumerate(GB[4:]):
                        q0 = _trim(kb - 12) if kb >= 12 else 0
                        nc.tensor.matmul(
                            pzB[:, q0:QB],
                            xkd_bf[:, kb * D + dsl * 128: kb * D + dsl * 128 + 128],
                            p_bf[:, (12 + j) * QB + q0:(13 + j) * QB],
                            start=False, stop=(j == 7))
                    nc.vector.scalar_tensor_tensor(
                        zb[:, dsl * QB:(dsl + 1) * QB],
                        zf[:, dsl * QB:(dsl + 1) * QB],
                        ws[:, 1:2], pzB[:], MUL, ADD)

                # ------- phase B3: O = Wv Z, A/B interleaved -------
                for dv in range(ND):
                    tag = "w5" if dv % 2 == 0 else "z"
                    po0 = psu.tile([128, QB], fp32, tag=tag, bufs=2)
                    po1 = psu.tile([128, QB], fp32, tag=tag, bufs=2)
                    for a in range(ND):
                        wv_ap = wv_bf[:, a * D + dv * 128: a * D + dv * 128 + 128]
                        nc.tensor.matmul(
                            po0[:], wv_ap, za[:, a * QB:(a + 1) * QB],
                            start=(a == 0), stop=(a == ND - 1))
                        nc.tensor.matmul(
                            po1[:], wv_ap, zb[:, a * QB:(a + 1) * QB],
                            start=(a == 0), stop=(a == ND - 1))
                    ot0 = op.tile([128, QB], fp32, tag="ot")
                    nc.vector.tensor_copy(ot0[:], po0[:])
                    nc.scalar.dma_start(outT_r[dv][:, 0:QB], ot0[:])
                    ot1 = op.tile([128, QB], fp32, tag="ot")
                    nc.vector.tensor_copy(ot1[:], po1[:])
                    nc.scalar.dma_start(outT_r[dv][:, QB:2 * QB], ot1[:])

    nc.compile()
    return nc


_NC_CACHE = {}


def _get_nc(body_reps=1):
    if body_reps not in _NC_CACHE:
        _NC_CACHE[body_reps] = build_nc(body_reps)
    return _NC_CACHE[body_reps]


def make_in_maps(x, Wq, Wk, Wv):
    x = np.asarray(x, dtype=np.float32)
    gTn = np.ascontiguousarray(
        np.asarray(Wq, np.float64).T @ np.asarray(Wk, np.float64)
    ).astype(ml_dtypes.bfloat16)
    wvT = np.ascontiguousarray(np.asarray(Wv, np.float32).T).astype(ml_dtypes.bfloat16)

    in_maps = []
    for core in range(N_CORES):
        b, qbs, gather, wa = _core_layout(core)
        xg = x[b][gather]
        xkdv = np.ascontiguousarray(xg).astype(ml_dtypes.bfloat16)
        xTp = np.ascontiguousarray(xg.T).astype(ml_dtypes.bfloat16)
        mkd = np.zeros((8, KB, QB), np.float32)
        for i in range(4):
            krows = gather[i * KB:(i + 1) * KB][:, None]
            qrows = (qbs[0] * QB + np.arange(QB))[None, :]
            mkd[i] = (krows <= qrows)
        for j in range(4):
            krows = gather[(12 + j) * KB:(13 + j) * KB][:, None]
            qrows = (qbs[1] * QB + np.arange(QB))[None, :]
            mkd[4 + j] = (krows <= qrows)
        wsel = np.zeros((KB, 2), np.float32)
        wsel[:, 0] = wa
        wsel[:, 1] = 1.0 - wa
        wlsel = np.zeros((2, QB), np.float32)
        wlsel[0, :] = wa
        wlsel[1, :] = 1.0 - wa
        in_maps.append({
            "xT": xTp,
            "xkd": xkdv,
            "gT": gTn,
            "wvT": wvT,
            "masks": mkd.astype(ml_dtypes.bfloat16),
            "wsel": wsel,
            "wlsel": wlsel,
        })
    return in_maps


def assemble_output(results):
    out = np.zeros((B, S, D), np.float32)
    for core in range(N_CORES):
        b, qbs, _, _ = _core_layout(core)
        outT = results[core]["outT"]
        l = results[core]["lsum"]
        for slot in range(2):
            rows = np.arange(qbs[slot] * QB, (qbs[slot] + 1) * QB)
            o = outT[:, slot * QB:(slot + 1) * QB].T
            out[b, rows, :] = o / l[slot][:, None]
    return out


def kernel(x, Wq, Wk, Wv):
    from concourse.bass_utils import run_bass_kernel_spmd
    nc = _get_nc()
    in_maps = make_in_maps(x, Wq, Wk, Wv)
    res = run_bass_kernel_spmd(nc, in_maps, core_ids=list(range(N_CORES)))
    return assemble_output(res.results)
